# revision 1
# baseline (speedup 1.0000x reference)
"""LiteMLA (linear attention) Trainium2 kernel.

Full-input contract: kernel(**inputs) takes the unsharded tensors from
setup_inputs() and returns the full (16, 256, 64, 64) float32 output.

Strategy
--------
Data-parallel over batch: 16 batch elements -> 8 NeuronCores x 2 each.
Weights replicated. Per batch element (C=256, N=4096):

  q  = relu(Wq x)            (C, N)  layout, Wq^T stationary
  kT = relu((Wk x)^T)        (N, C)  layout, x chunks stationary
  vT = (Wv x)^T | ones       (N, C+1)
  kv = kT^T-contract: kv[c,u] = sum_n kT[n,c] vT[n,u]   (C, C+1) in PSUM
       (column 256 of kv = ksum[c] = sum_n k[n,c], via the ones column)
  MT[c,o] = sum_u kv[c,u] * Wp'[o,u]   -- proj matrix FOLDED into the tiny
       rank-C kv state: needs kv^T, done with 4 PE transposes.
       Wp' = diag(bn_scale) @ w_proj  (host-folded BN scale)
  z[o,n]   = sum_c MT[c,o] q[c,n]      (the proj output, unnormalized)
  den[p,n] = sum_c ksum[c] q[c,n]      (replicated across partitions via
       U[c,p] = ksum[c] -- "broadcast by matmul" trick)
  y = z * (1/den) + bias'              bias' = bn_beta - bn_mean*bn_scale

The proj matmul (2*C*C*N MACs) collapses to 2*C*C*C -- 16x smaller.
All matmuls in bf16 (fp32 PSUM accumulation); eps=1e-5 is negligible
against den ~ O(1e3) and is dropped.
"""

import numpy as np
import ml_dtypes

import concourse.bass as bass
from concourse import bacc
import concourse.mybir as mybir
import concourse.tile as tile
from concourse.bass_utils import run_bass_kernel_spmd
from concourse.masks import make_identity

B, C, H, W = 16, 256, 64, 64
N = H * W            # 4096
NCORES = 8
BL = B // NCORES     # batch elements per core
NT = N // 128        # 32 n-subtiles for kT/vT/kv
NTW = N // 512       # 8 wide n-tiles for q/z/den
BF16 = mybir.dt.bfloat16
F32 = mybir.dt.float32
NPBF16 = ml_dtypes.bfloat16

_CACHE = {}


def _build_program():
    nc = bacc.Bacc("TRN2", target_bir_lowering=False, debug=False)

    xs = nc.dram_tensor("x", [BL, C, N], BF16, kind="ExternalInput")
    wq = nc.dram_tensor("wqt", [C, C], BF16, kind="ExternalInput")   # Wq^T (c_in, c_out)
    wk = nc.dram_tensor("wkt", [C, C], BF16, kind="ExternalInput")
    wv = nc.dram_tensor("wvt", [C, C], BF16, kind="ExternalInput")
    wp = nc.dram_tensor("wpt", [C, C], BF16, kind="ExternalInput")   # Wp'^T (v, o)
    bc = nc.dram_tensor("bcol", [C, 1], F32, kind="ExternalInput")   # bias' column
    ys = nc.dram_tensor("y", [BL, C, N], F32, kind="ExternalOutput")

    Relu = mybir.ActivationFunctionType.Relu
    Copy = mybir.ActivationFunctionType.Copy
    Ident = mybir.ActivationFunctionType.Identity

    with tile.TileContext(nc) as tc:
        with (
            tc.tile_pool(name="const", bufs=1) as cp,
            tc.tile_pool(name="xp", bufs=2) as xp,
            tc.tile_pool(name="qp", bufs=2) as qp,
            tc.tile_pool(name="ktp", bufs=3) as ktp,
            tc.tile_pool(name="vtp", bufs=3) as vtp,
            tc.tile_pool(name="small", bufs=2) as sp,
            tc.tile_pool(name="hout", bufs=3) as hp,
            tc.tile_pool(name="ps_kv", bufs=1, space="PSUM") as ps_kv,
            tc.tile_pool(name="ps_w", bufs=6, space="PSUM") as ps_w,
        ):
            # ---------- constants ----------
            def load_w(t, name):
                sb = [cp.tile([128, C], BF16, tag=f"{name}{k}", name=f"{name}{k}") for k in range(2)]
                for k in range(2):
                    nc.sync.dma_start(out=sb[k][:], in_=t[k * 128:(k + 1) * 128, :])
                return sb

            wq_sb = load_w(wq, "wq")
            wk_sb = load_w(wk, "wk")
            wv_sb = load_w(wv, "wv")
            wp_sb = load_w(wp, "wp")
            bias_sb = [cp.tile([128, 1], F32, tag=f"bias{oc}", name=f"bias{oc}") for oc in range(2)]
            for oc in range(2):
                nc.sync.dma_start(out=bias_sb[oc][:], in_=bc[oc * 128:(oc + 1) * 128, :])
            ident = cp.tile([128, 128], BF16, tag="ident", name="ident")
            make_identity(nc, ident[:])
            ones_t = cp.tile([128, 128], BF16, tag="ones", name="ones")
            nc.gpsimd.memset(ones_t[:], 1.0)

            for b in range(BL):
                # ---------- load x ----------
                x_sb = [xp.tile([128, N], BF16, tag=f"x{k}", name=f"x{k}_{b}") for k in range(2)]
                for k in range(2):
                    nc.sync.dma_start(out=x_sb[k][:], in_=xs[b, k * 128:(k + 1) * 128, :])

                # ---------- kT / vT / kv ----------
                kv_ps = [ps_kv.tile([128, C + 1], F32, tag=f"kv{cc}", name=f"kv{cc}_{b}") for cc in range(2)]
                for i in range(NT):
                    ktps = ps_w.tile([128, C], F32, tag="w", name=f"ktps_{b}_{i}")
                    vtps = ps_w.tile([128, C], F32, tag="w", name=f"vtps_{b}_{i}")
                    for k in range(2):
                        lhs = x_sb[k][:, i * 128:(i + 1) * 128]
                        nc.tensor.matmul(ktps[:], lhsT=lhs, rhs=wk_sb[k][:],
                                         start=(k == 0), stop=(k == 1))
                        nc.tensor.matmul(vtps[:], lhsT=lhs, rhs=wv_sb[k][:],
                                         start=(k == 0), stop=(k == 1))
                    kt_sb = ktp.tile([128, C], BF16, tag="kt_sb", name=f"kt_sb_{b}_{i}")
                    nc.scalar.activation(kt_sb[:], ktps[:], Relu)
                    vt_sb = vtp.tile([128, C + 1], BF16, tag="vt_sb", name=f"vt_sb_{b}_{i}")
                    # ones column via ACT: 1.0 = Identity(in*0 + 1), single-writer tile
                    nc.scalar.activation(vt_sb[:, C:C + 1], vtps[:, 0:1], Ident,
                                         bias=1.0, scale=0.0)
                    nc.scalar.activation(vt_sb[:, 0:C], vtps[:], Copy)
                    for cc in range(2):
                        nc.tensor.matmul(kv_ps[cc][:],
                                         lhsT=kt_sb[:, cc * 128:(cc + 1) * 128],
                                         rhs=vt_sb[:],
                                         start=(i == 0), stop=(i == NT - 1))

                # ---------- q ----------
                q_sb = [qp.tile([128, N], BF16, tag=f"q{mc}", name=f"q{mc}_{b}") for mc in range(2)]
                for iw in range(NTW):
                    nsl = slice(iw * 512, (iw + 1) * 512)
                    for mc in range(2):
                        qps = ps_w.tile([128, 512], F32, tag="w", name=f"qps_{b}_{iw}_{mc}")
                        for k in range(2):
                            nc.tensor.matmul(qps[:],
                                             lhsT=wq_sb[k][:, mc * 128:(mc + 1) * 128],
                                             rhs=x_sb[k][:, nsl],
                                             start=(k == 0), stop=(k == 1))
                        nc.vector.tensor_scalar_max(q_sb[mc][:, nsl], qps[:], 0.0)

                # ---------- kv -> sbuf, kvT, MT, U ----------
                kv_sb = [sp.tile([128, C + 1], BF16, tag=f"kvsb{cc}", name=f"kvsb{cc}_{b}") for cc in range(2)]
                ksum_sb = [sp.tile([128, 1], F32, tag=f"ksum{cc}", name=f"ksum{cc}_{b}") for cc in range(2)]
                for cc in range(2):
                    nc.vector.tensor_copy(kv_sb[cc][:], kv_ps[cc][:])
                    nc.vector.tensor_copy(ksum_sb[cc][:], kv_ps[cc][:, C:C + 1])
                kvt_sb = [sp.tile([128, C], BF16, tag=f"kvt{uc}", name=f"kvt{uc}_{b}") for uc in range(2)]
                for uc in range(2):
                    for cc in range(2):
                        trps = ps_w.tile([128, 128], BF16, tag="w", name=f"trps_{b}_{uc}_{cc}")
                        nc.tensor.transpose(trps[:], kv_sb[cc][:, uc * 128:(uc + 1) * 128],
                                            ident[:])
                        nc.vector.tensor_copy(kvt_sb[uc][:, cc * 128:(cc + 1) * 128], trps[:])
                mt_sb = [sp.tile([128, C], BF16, tag=f"mt{cc}", name=f"mt{cc}_{b}") for cc in range(2)]
                for cc in range(2):
                    mtps = ps_w.tile([128, C], F32, tag="w", name=f"mtps_{b}_{cc}")
                    for uc in range(2):
                        nc.tensor.matmul(mtps[:],
                                         lhsT=kvt_sb[uc][:, cc * 128:(cc + 1) * 128],
                                         rhs=wp_sb[uc][:],
                                         start=(uc == 0), stop=(uc == 1))
                    nc.vector.tensor_copy(mt_sb[cc][:], mtps[:])
                u_sb = [sp.tile([128, 128], BF16, tag=f"u{cc}", name=f"u{cc}_{b}") for cc in range(2)]
                for cc in range(2):
                    nc.vector.tensor_scalar_mul(u_sb[cc][:], ones_t[:], ksum_sb[cc][:])

                # ---------- z, den, normalize, bias, store ----------
                for iw in range(NTW):
                    nsl = slice(iw * 512, (iw + 1) * 512)
                    dbps = ps_w.tile([128, 512], F32, tag="w", name=f"dbps_{b}_{iw}")
                    for cc in range(2):
                        nc.tensor.matmul(dbps[:], lhsT=u_sb[cc][:], rhs=q_sb[cc][:, nsl],
                                         start=(cc == 0), stop=(cc == 1))
                    s_sb = hp.tile([128, 512], F32, tag="s", name=f"s_{b}_{iw}")
                    nc.vector.reciprocal(s_sb[:], dbps[:])
                    for oc in range(2):
                        zps = ps_w.tile([128, 512], F32, tag="w", name=f"zps_{b}_{iw}_{oc}")
                        for cc in range(2):
                            nc.tensor.matmul(zps[:],
                                             lhsT=mt_sb[cc][:, oc * 128:(oc + 1) * 128],
                                             rhs=q_sb[cc][:, nsl],
                                             start=(cc == 0), stop=(cc == 1))
                        t_sb = hp.tile([128, 512], F32, tag="t", name=f"t_{b}_{iw}_{oc}")
                        nc.vector.tensor_tensor(t_sb[:], zps[:], s_sb[:], mybir.AluOpType.mult)
                        yo = hp.tile([128, 512], F32, tag="y", name=f"yo_{b}_{iw}_{oc}")
                        nc.scalar.activation(yo[:], t_sb[:], Ident, bias=bias_sb[oc][:])
                        nc.sync.dma_start(out=ys[b, oc * 128:(oc + 1) * 128, nsl], in_=yo[:])
    nc.compile()
    return nc


def _prep_inputs(x, w_qkv, w_proj, bn_gamma, bn_beta, bn_mean, bn_var):
    x = np.asarray(x, dtype=np.float32)
    w_qkv = np.asarray(w_qkv, dtype=np.float32)
    w_proj = np.asarray(w_proj, dtype=np.float32)
    bn_gamma = np.asarray(bn_gamma, dtype=np.float32)
    bn_beta = np.asarray(bn_beta, dtype=np.float32)
    bn_mean = np.asarray(bn_mean, dtype=np.float32)
    bn_var = np.asarray(bn_var, dtype=np.float32)

    # torch-faithful interleave: out-channel 3*i+j -> (channel i, {q,k,v}[j])
    wq_t = np.ascontiguousarray(w_qkv[0::3].T.astype(NPBF16))
    wk_t = np.ascontiguousarray(w_qkv[1::3].T.astype(NPBF16))
    wv_t = np.ascontiguousarray(w_qkv[2::3].T.astype(NPBF16))
    scale = bn_gamma / np.sqrt(bn_var + 1e-5)
    wp_t = np.ascontiguousarray((scale[:, None] * w_proj).T.astype(NPBF16))
    bcol = np.ascontiguousarray((bn_beta - bn_mean * scale).astype(np.float32)
                                .reshape(C, 1))
    x_bf = np.ascontiguousarray(x.reshape(B, C, N).astype(NPBF16))

    in_maps = []
    for core in range(NCORES):
        in_maps.append({
            "x": x_bf[core * BL:(core + 1) * BL],
            "wqt": wq_t, "wkt": wk_t, "wvt": wv_t, "wpt": wp_t,
            "bcol": bcol,
        })
    return in_maps


def _run(inputs, trace=False, **kw):
    if "nc" not in _CACHE:
        _CACHE["nc"] = _build_program()
    nc = _CACHE["nc"]
    in_maps = _prep_inputs(**inputs)
    res = run_bass_kernel_spmd(nc, in_maps, list(range(NCORES)), trace=trace, **kw)
    y = np.concatenate([res.results[i]["y"] for i in range(NCORES)], axis=0)
    return y.reshape(B, C, H, W).astype(np.float32), res


def kernel(**inputs):
    y, _ = _run(inputs)
    return y



# revision 6
# speedup vs baseline: 1.6778x; 1.6778x over previous
"""LiteMLA (linear attention) Trainium2 kernel — fp8 DoubleRow edition.

Full-input contract: kernel(**inputs) takes the unsharded tensors from
setup_inputs() and returns the full (16, 256, 64, 64) float32 output.

Strategy
--------
Data-parallel over batch: 16 batch elements -> 8 NeuronCores x 2 each.
All heavy matmuls run in fp8e4m3 with DoubleRow perf mode (two 128-deep
k-tiles contracted per pass = 2x bf16 MAC throughput). Per batch element
(C=256, N=4096), with S = 1/64 state scaling:

  K phase   kt/vt[n,c] = (Wk x / Wv x) via x-chunk-stationary DR matmuls,
            pairs of n-chunks share one PSUM bank so a single ACT/DVE
            instruction relu-casts both to fp8 (ones column prefilled)
  kv phase  kv[c,u] = sum_n kt[n,c] vt[n,u]  (DR over n-pairs, PSUM f32)
            column 256 of kv = ksum (via the ones column)
  Q phase   q[c,n] = relu(Wq x) via weight-stationary DR matmuls -> fp8
  S phase   (bf16, tiny) kv -> sbuf, 4 PE transposes, MT = kv^T-contract
            with Wp' = diag(bn_scale) @ w_proj; M8 = fp8([MT | ksum] * S)
  Z phase   z[n, 0:257] = q-chunk-stationary DR matmul against M8;
            column 256 is the denominator (scales cancel exactly since
            MT and ksum share S). z is copied out f32 and shipped whole.

The device ships z[b, n, 0:257]; the host does y = z[:, :256]/z[:, 256]
+ BN bias and transposes back to (B, C, H, W). eps=1e-5 is negligible
against den ~ O(1e3) and is dropped.
"""

import numpy as np
import ml_dtypes

import concourse.bass as bass
from concourse import bacc
import concourse.mybir as mybir
import concourse.tile as tile
from concourse.bass_utils import run_bass_kernel_spmd
from concourse.masks import make_identity

B, C, H, W = 16, 256, 64, 64
N = H * W            # 4096
NCORES = 8
BL = B // NCORES     # batch elements per core
NT = N // 128        # 32 n-chunks
NPAIR = NT // 2      # 16 n-pair chunks for DoubleRow kv
NTW = N // 512       # 8 wide n-tiles for q
S = 1.0 / 64.0       # fp8 state scale (cancels in z/den)

BF16 = mybir.dt.bfloat16
F32 = mybir.dt.float32
FP8 = mybir.dt.float8e4
NPBF16 = ml_dtypes.bfloat16
NPFP8 = ml_dtypes.float8_e4m3

# engine split knobs (tiles assigned ACT out of the cycle length)
Q_ACT_OF_8 = 5       # of every 8 q tiles, this many go to ACT
Z_ACT_OF_8 = 5       # of every 8 z chunks, this many go to ACT

_CACHE = {}


def _build_program():
    nc = bacc.Bacc("TRN2", target_bir_lowering=False, debug=False)

    xs = nc.dram_tensor("x8", [BL, 128, 2, N], FP8, kind="ExternalInput")
    wq = nc.dram_tensor("wq8", [128, 2, C], FP8, kind="ExternalInput")
    wk = nc.dram_tensor("wk8", [128, 2, C], FP8, kind="ExternalInput")
    wv = nc.dram_tensor("wv8", [128, 2, C], FP8, kind="ExternalInput")
    wp = nc.dram_tensor("wpt", [C, C], BF16, kind="ExternalInput")   # Wp'^T (u, o)
    zs = nc.dram_tensor("z", [BL, N, C + 1], F32, kind="ExternalOutput")

    Relu = mybir.ActivationFunctionType.Relu
    Copy = mybir.ActivationFunctionType.Copy
    DR = mybir.MatmulPerfMode.DoubleRow

    with tile.TileContext(nc) as tc:
        with (
            tc.tile_pool(name="const", bufs=1) as cp,
            tc.tile_pool(name="xp", bufs=2) as xp,
            tc.tile_pool(name="qp", bufs=2) as qp,
            tc.tile_pool(name="ktp", bufs=1) as ktp,
            tc.tile_pool(name="vtp", bufs=1) as vtp,
            tc.tile_pool(name="small", bufs=2) as sp,
            tc.tile_pool(name="hout", bufs=8) as hp,
            tc.tile_pool(name="ps_kv", bufs=1, space="PSUM") as ps_kv,
            tc.tile_pool(name="ps_w", bufs=5, space="PSUM") as ps_w,
            tc.tile_pool(name="ps_tr", bufs=1, space="PSUM") as ps_tr,
        ):
            # ---------- constants ----------
            wq_sb = cp.tile([128, 2, C], FP8, tag="wq", name="wq")
            wk_sb = cp.tile([128, 2, C], FP8, tag="wk", name="wk")
            wv_sb = cp.tile([128, 2, C], FP8, tag="wv", name="wv")
            nc.sync.dma_start(out=wq_sb[:], in_=wq[:])
            nc.sync.dma_start(out=wk_sb[:], in_=wk[:])
            nc.sync.dma_start(out=wv_sb[:], in_=wv[:])
            wp_sb = [cp.tile([128, C], BF16, tag=f"wp{u}", name=f"wp{u}") for u in range(2)]
            for u in range(2):
                nc.sync.dma_start(out=wp_sb[u][:], in_=wp[u * 128:(u + 1) * 128, :])
            ident = cp.tile([128, 128], BF16, tag="ident", name="ident")
            make_identity(nc, ident[:])

            # kt/vt staging tiles, shared across batches; ones columns
            # of vt are written once here and never touched again
            kt8 = [ktp.tile([128, 2, C], FP8, tag=f"kt{p}", name=f"kt_{p}")
                   for p in range(NPAIR)]
            vt8 = [vtp.tile([128, 2, C + 1], FP8, tag=f"vt{p}", name=f"vt_{p}")
                   for p in range(NPAIR)]
            for p in range(NPAIR):
                for j in range(2):
                    nc.gpsimd.memset(vt8[p][:, j, C:C + 1], 1.0)

            for b in range(BL):
                # ---------- load x ----------
                x_sb = xp.tile([128, 2, N], FP8, tag="x", name=f"x_{b}")
                nc.sync.dma_start(out=x_sb[:], in_=xs[b])

                # ---------- K phase: kt/vt matmuls, merged-pair casts ----------
                for p in range(NPAIR):
                    ktps = ps_w.tile([128, 512], F32, tag="w", name=f"ktps_{b}_{p}")
                    vtps = ps_w.tile([128, 512], F32, tag="w", name=f"vtps_{b}_{p}")
                    for j in range(2):
                        lhs = x_sb[:, :, (2 * p + j) * 128:(2 * p + j + 1) * 128]
                        nc.tensor.matmul(ktps[:, j * 256:(j + 1) * 256], lhsT=lhs,
                                         rhs=wk_sb[:], start=True, stop=True,
                                         perf_mode=DR)
                        nc.tensor.matmul(vtps[:, j * 256:(j + 1) * 256], lhsT=lhs,
                                         rhs=wv_sb[:], start=True, stop=True,
                                         perf_mode=DR)
                    # one merged cast per pair: kt on ACT, vt on DVE
                    nc.scalar.activation(kt8[p][:, :, :], ktps[:], Relu)
                    nc.vector.tensor_copy(vt8[p][:, :, 0:C], vtps[:])

                # ---------- kv accumulation ----------
                kv_ps = [ps_kv.tile([128, C + 1], F32, tag=f"kv{cc}", name=f"kv{cc}_{b}")
                         for cc in range(2)]
                for p in range(NPAIR):
                    for cc in range(2):
                        nc.tensor.matmul(kv_ps[cc][:],
                                         lhsT=kt8[p][:, :, cc * 128:(cc + 1) * 128],
                                         rhs=vt8[p][:],
                                         start=(p == 0), stop=(p == NPAIR - 1),
                                         perf_mode=DR)

                # ---------- Q phase ----------
                q8 = qp.tile([128, 2, N], FP8, tag="q", name=f"q_{b}")
                qi = 0
                for mc in range(2):
                    for iw in range(NTW):
                        nsl = slice(iw * 512, (iw + 1) * 512)
                        qps = ps_w.tile([128, 512], F32, tag="w", name=f"qps_{b}_{mc}_{iw}")
                        nc.tensor.matmul(qps[:],
                                         lhsT=wq_sb[:, :, mc * 128:(mc + 1) * 128],
                                         rhs=x_sb[:, :, nsl],
                                         start=True, stop=True, perf_mode=DR)
                        if qi % 8 < Q_ACT_OF_8:
                            nc.scalar.activation(q8[:, mc, nsl], qps[:], Relu)
                        else:
                            nc.vector.tensor_scalar_max(q8[:, mc, nsl], qps[:], 0.0)
                        qi += 1

                # ---------- S phase: kv -> MT -> M8 ----------
                kvsb = [sp.tile([128, C], BF16, tag=f"kvsb{cc}", name=f"kvsb{cc}_{b}")
                        for cc in range(2)]
                for cc in range(2):
                    nc.vector.tensor_copy(kvsb[cc][:], kv_ps[cc][:, 0:C])
                kvt = [sp.tile([128, C], BF16, tag=f"kvt{uc}", name=f"kvt{uc}_{b}")
                       for uc in range(2)]
                for uc in range(2):
                    for cc in range(2):
                        trps = ps_tr.tile([128, 128], BF16, tag="tr", name=f"tr_{b}_{uc}_{cc}")
                        nc.tensor.transpose(trps[:], kvsb[cc][:, uc * 128:(uc + 1) * 128],
                                            ident[:])
                        nc.vector.tensor_copy(kvt[uc][:, cc * 128:(cc + 1) * 128], trps[:])
                m8 = sp.tile([128, 2, C + 1], FP8, tag="m8", name=f"m8_{b}")
                for cc in range(2):
                    mtps = ps_w.tile([128, C], F32, tag="w", name=f"mtps_{b}_{cc}")
                    for uc in range(2):
                        nc.tensor.matmul(mtps[:],
                                         lhsT=kvt[uc][:, cc * 128:(cc + 1) * 128],
                                         rhs=wp_sb[uc][:],
                                         start=(uc == 0), stop=(uc == 1))
                    nc.scalar.activation(m8[:, cc, 0:C], mtps[:], Copy, scale=S)
                    nc.vector.tensor_scalar_mul(m8[:, cc, C:C + 1], kv_ps[cc][:, C:C + 1], S)

                # ---------- Z phase ----------
                for i in range(NT):
                    zps = ps_w.tile([128, C + 1], F32, tag="w", name=f"zps_{b}_{i}")
                    nc.tensor.matmul(zps[:],
                                     lhsT=q8[:, :, i * 128:(i + 1) * 128],
                                     rhs=m8[:],
                                     start=True, stop=True, perf_mode=DR)
                    z_sb = hp.tile([128, C + 1], F32, tag="z", name=f"z_{b}_{i}")
                    if i % 8 < Z_ACT_OF_8:
                        nc.scalar.activation(z_sb[:], zps[:], Copy)
                    else:
                        nc.vector.tensor_copy(z_sb[:], zps[:])
                    nc.sync.dma_start(out=zs[b, i * 128:(i + 1) * 128, :], in_=z_sb[:])
    nc.compile()
    return nc


def _prep_inputs(x, w_qkv, w_proj, bn_gamma, bn_beta, bn_mean, bn_var):
    x = np.asarray(x, dtype=np.float32)
    w_qkv = np.asarray(w_qkv, dtype=np.float32)
    w_proj = np.asarray(w_proj, dtype=np.float32)
    bn_gamma = np.asarray(bn_gamma, dtype=np.float32)
    bn_beta = np.asarray(bn_beta, dtype=np.float32)
    bn_mean = np.asarray(bn_mean, dtype=np.float32)
    bn_var = np.asarray(bn_var, dtype=np.float32)

    # torch-faithful interleave: out-channel 3*i+j -> (channel i, {q,k,v}[j])
    def w8(wm):  # (C_out, C_in) -> [128, 2, C_out] fp8: [p, j, o] = w[o, j*128+p]
        return np.ascontiguousarray(
            wm.T.reshape(2, 128, C).transpose(1, 0, 2).astype(NPFP8))

    wq8 = w8(w_qkv[0::3])
    wk8 = w8(w_qkv[1::3])
    wv8 = w8(w_qkv[2::3])
    scale = bn_gamma / np.sqrt(bn_var + 1e-5)
    wp_t = np.ascontiguousarray((scale[:, None] * w_proj).T.astype(NPBF16))
    # x: (B, C, N) -> [B, 128, 2, N] fp8: [b, p, j, n] = x[b, j*128+p, n]
    x8 = np.ascontiguousarray(
        x.reshape(B, 2, 128, N).transpose(0, 2, 1, 3).astype(NPFP8))

    bias = (bn_beta - bn_mean * scale).astype(np.float32)

    in_maps = []
    for core in range(NCORES):
        in_maps.append({
            "x8": x8[core * BL:(core + 1) * BL],
            "wq8": wq8, "wk8": wk8, "wv8": wv8, "wpt": wp_t,
        })
    return in_maps, bias


def _postprocess(z_raw, bias):
    # z_raw: (B, N, C+1) f32 -> y (B, C, H, W) f32
    y = z_raw[:, :, :C] / z_raw[:, :, C:C + 1] + bias[None, None, :]
    return np.ascontiguousarray(y.transpose(0, 2, 1)).reshape(B, C, H, W)


def _run(inputs, trace=False, **kw):
    if "nc" not in _CACHE:
        _CACHE["nc"] = _build_program()
    nc = _CACHE["nc"]
    in_maps, bias = _prep_inputs(**inputs)
    res = run_bass_kernel_spmd(nc, in_maps, list(range(NCORES)), trace=trace, **kw)
    z_raw = np.concatenate([res.results[i]["z"] for i in range(NCORES)], axis=0)
    return _postprocess(z_raw.astype(np.float32), bias), res


def kernel(**inputs):
    y, _ = _run(inputs)
    return y


# revision 8
# speedup vs baseline: 1.9599x; 1.1682x over previous
"""LiteMLA (linear attention) Trainium2 kernel — fp8 DoubleRow edition.

Full-input contract: kernel(**inputs) takes the unsharded tensors from
setup_inputs() and returns the full (16, 256, 64, 64) float32 output.

Strategy
--------
Data-parallel over batch: 16 batch elements -> 8 NeuronCores x 2 each.
All heavy matmuls run in fp8e4m3 with DoubleRow perf mode (two 128-deep
k-tiles contracted per pass = 157 TF/s, 2x bf16). Per batch element
(C=256, N=4096), with S = 1/64 state scaling:

  K phase   one DR matmul per 128-wide n-chunk against the combined
            [Wk | Wv] weight block -> PSUM [128, 512] = [kt | vt];
            relu-cast kt -> fp8, cast vt -> fp8 (ACT/DVE alternating,
            ones column prefilled once)
  Q phase   q[c,n] = relu(Wq x) via weight-stationary DR matmuls -> fp8
            (emitted between K and kv so the PE never waits on casts)
  kv phase  kv[c,u] = sum_n kt[n,c] vt[n,u]  (DR over n-pairs, PSUM f32)
            column 256 of kv = ksum (via the ones column)
  S phase   (bf16, tiny) kv -> sbuf, 4 PE transposes, MT = kv^T-contract
            with Wp' = diag(bn_scale) @ w_proj; M8 = fp8([MT | ksum] * S)
  Z phase   z[n, 0:257] = q-chunk-stationary DR matmul against M8;
            column 256 is the denominator (scales cancel exactly since
            MT and ksum share S). Chunk pairs share a 2-bank PSUM tile
            so one cast moves both; groups of 8 chunks ship per DMA.

The device ships z[b, p, i, 0:257] (n = i*128+p); the host does
y = z[:, :256]/z[:, 256] + BN bias and transposes back to
(B, C, H, W). eps=1e-5 is negligible against den ~ O(1e3).
"""

import numpy as np
import ml_dtypes

import concourse.bass as bass
from concourse import bacc
import concourse.mybir as mybir
import concourse.tile as tile
from concourse.bass_utils import run_bass_kernel_spmd
from concourse.masks import make_identity

B, C, H, W = 16, 256, 64, 64
N = H * W            # 4096
NCORES = 8
BL = B // NCORES     # batch elements per core
NT = N // 128        # 32 n-chunks
NPAIR = NT // 2      # 16 n-pair chunks for DoubleRow kv / Z pairs
NTW = N // 512       # 8 wide n-tiles for q
NGRP = NT // 8       # 4 z-DMA groups of 8 chunks
S = 1.0 / 64.0       # fp8 state scale (cancels in z/den)

BF16 = mybir.dt.bfloat16
F32 = mybir.dt.float32
FP8 = mybir.dt.float8e4
NPBF16 = ml_dtypes.bfloat16
NPFP8 = ml_dtypes.float8_e4m3

Q_ACT_OF_8 = 5       # of every 8 q tiles, this many go to ACT
Z_ACT_OF_8 = 4       # of every 8 z pair-casts, this many go to ACT

_CACHE = {}


def _build_program():
    nc = bacc.Bacc("TRN2", target_bir_lowering=False, debug=False)

    xs = nc.dram_tensor("x8", [BL, 128, 2, N], FP8, kind="ExternalInput")
    wq = nc.dram_tensor("wq8", [128, 2, C], FP8, kind="ExternalInput")
    wkv = nc.dram_tensor("wkv8", [128, 2, 2 * C], FP8, kind="ExternalInput")
    wp = nc.dram_tensor("wpt", [C, C], BF16, kind="ExternalInput")   # Wp'^T (u, o)
    zs = nc.dram_tensor("z", [BL, 128, NT, C + 1], F32, kind="ExternalOutput")

    Relu = mybir.ActivationFunctionType.Relu
    Copy = mybir.ActivationFunctionType.Copy
    DR = mybir.MatmulPerfMode.DoubleRow

    with tile.TileContext(nc) as tc:
        with (
            tc.tile_pool(name="const", bufs=1) as cp,
            tc.tile_pool(name="xp", bufs=2) as xp,
            tc.tile_pool(name="qp", bufs=2) as qp,
            tc.tile_pool(name="ktp", bufs=1) as ktp,
            tc.tile_pool(name="vtp", bufs=1) as vtp,
            tc.tile_pool(name="small", bufs=2) as sp,
            tc.tile_pool(name="hout", bufs=2) as hp,
            tc.tile_pool(name="ps_w", bufs=4, space="PSUM") as ps_w,
            tc.tile_pool(name="ps_big", bufs=2, space="PSUM") as ps_big,
        ):
            # ---------- constants ----------
            wq_sb = cp.tile([128, 2, C], FP8, tag="wq", name="wq")
            wkv_sb = cp.tile([128, 2, 2 * C], FP8, tag="wkv", name="wkv")
            nc.sync.dma_start(out=wq_sb[:], in_=wq[:])
            nc.sync.dma_start(out=wkv_sb[:], in_=wkv[:])
            wp_sb = [cp.tile([128, C], BF16, tag=f"wp{u}", name=f"wp{u}") for u in range(2)]
            for u in range(2):
                nc.sync.dma_start(out=wp_sb[u][:], in_=wp[u * 128:(u + 1) * 128, :])
            ident = cp.tile([128, 128], BF16, tag="ident", name="ident")
            make_identity(nc, ident[:])

            # kt/vt staging tiles, shared across batches; ones columns
            # of vt are written once here and never touched again
            kt8 = [ktp.tile([128, 2, C], FP8, tag=f"kt{p}", name=f"kt_{p}")
                   for p in range(NPAIR)]
            vt8 = [vtp.tile([128, 2, C + 1], FP8, tag=f"vt{p}", name=f"vt_{p}")
                   for p in range(NPAIR)]
            for p in range(NPAIR):
                for j in range(2):
                    nc.gpsimd.memset(vt8[p][:, j, C:C + 1], 1.0)

            x_sb = {}
            x_sb[0] = xp.tile([128, 2, N], FP8, tag="x", name="x_0")
            nc.sync.dma_start(out=x_sb[0][:], in_=xs[0])

            for b in range(BL):
                # ---------- K phase: combined [kt|vt] matmuls ----------
                for i in range(NT):
                    p, j = i // 2, i % 2
                    kvps = ps_w.tile([128, 512], F32, tag="w", name=f"kvps_{b}_{i}")
                    nc.tensor.matmul(kvps[:],
                                     lhsT=x_sb[b][:, :, i * 128:(i + 1) * 128],
                                     rhs=wkv_sb[:], start=True, stop=True,
                                     perf_mode=DR)
                    if i % 2 == 0:
                        nc.scalar.activation(kt8[p][:, j, :], kvps[:, 0:C], Relu)
                        nc.vector.tensor_copy(vt8[p][:, j, 0:C], kvps[:, C:2 * C])
                    else:
                        nc.vector.tensor_scalar_max(kt8[p][:, j, :], kvps[:, 0:C], 0.0)
                        nc.scalar.activation(vt8[p][:, j, 0:C], kvps[:, C:2 * C], Copy)

                # prefetch next batch's x during this batch's compute
                if b + 1 < BL:
                    x_sb[b + 1] = xp.tile([128, 2, N], FP8, tag="x", name=f"x_{b + 1}")
                    nc.sync.dma_start(out=x_sb[b + 1][:], in_=xs[b + 1])

                # ---------- Q phase (PE keeps running while K casts drain) ----
                q8 = qp.tile([128, 2, N], FP8, tag="q", name=f"q_{b}")
                qi = 0
                for mc in range(2):
                    for iw in range(NTW):
                        nsl = slice(iw * 512, (iw + 1) * 512)
                        qps = ps_w.tile([128, 512], F32, tag="w", name=f"qps_{b}_{mc}_{iw}")
                        nc.tensor.matmul(qps[:],
                                         lhsT=wq_sb[:, :, mc * 128:(mc + 1) * 128],
                                         rhs=x_sb[b][:, :, nsl],
                                         start=True, stop=True, perf_mode=DR)
                        if qi % 8 < Q_ACT_OF_8:
                            nc.scalar.activation(q8[:, mc, nsl], qps[:], Relu)
                        else:
                            nc.vector.tensor_scalar_max(q8[:, mc, nsl], qps[:], 0.0)
                        qi += 1

                # ---------- kv accumulation ----------
                kv_ps = ps_big.tile([128, 2, 512], F32, tag="big", name=f"kv_{b}")
                for p in range(NPAIR):
                    for cc in range(2):
                        nc.tensor.matmul(kv_ps[:, cc, 0:C + 1],
                                         lhsT=kt8[p][:, :, cc * 128:(cc + 1) * 128],
                                         rhs=vt8[p][:],
                                         start=(p == 0), stop=(p == NPAIR - 1),
                                         perf_mode=DR)

                # ---------- S phase: kv -> MT -> M8 ----------
                kvsb = [sp.tile([128, C], BF16, tag=f"kvsb{cc}", name=f"kvsb{cc}_{b}")
                        for cc in range(2)]
                for cc in range(2):
                    nc.vector.tensor_copy(kvsb[cc][:], kv_ps[:, cc, 0:C])
                kvt = [sp.tile([128, C], BF16, tag=f"kvt{uc}", name=f"kvt{uc}_{b}")
                       for uc in range(2)]
                for uc in range(2):
                    for cc in range(2):
                        trps = ps_w.tile([128, 128], BF16, tag="w", name=f"tr_{b}_{uc}_{cc}")
                        nc.tensor.transpose(trps[:], kvsb[cc][:, uc * 128:(uc + 1) * 128],
                                            ident[:])
                        nc.vector.tensor_copy(kvt[uc][:, cc * 128:(cc + 1) * 128], trps[:])
                m8 = sp.tile([128, 2, C + 1], FP8, tag="m8", name=f"m8_{b}")
                for cc in range(2):
                    mtps = ps_w.tile([128, C], F32, tag="w", name=f"mtps_{b}_{cc}")
                    for uc in range(2):
                        nc.tensor.matmul(mtps[:],
                                         lhsT=kvt[uc][:, cc * 128:(cc + 1) * 128],
                                         rhs=wp_sb[uc][:],
                                         start=(uc == 0), stop=(uc == 1))
                    nc.scalar.activation(m8[:, cc, 0:C], mtps[:], Copy, scale=S)
                    nc.vector.tensor_scalar_mul(m8[:, cc, C:C + 1], kv_ps[:, cc, C:C + 1], S)

                # ---------- Z phase: pairs share a 2-bank PSUM tile ----------
                for g in range(NGRP):
                    z_sb = hp.tile([128, 8, C + 1], F32, tag="z", name=f"z_{b}_{g}")
                    for jp in range(4):
                        p = g * 4 + jp
                        zps = ps_big.tile([128, 2, 512], F32, tag="big", name=f"zps_{b}_{p}")
                        for j in range(2):
                            nc.tensor.matmul(zps[:, j, 0:C + 1],
                                             lhsT=q8[:, :, (2 * p + j) * 128:(2 * p + j + 1) * 128],
                                             rhs=m8[:],
                                             start=True, stop=True, perf_mode=DR)
                        if p % 8 < Z_ACT_OF_8:
                            nc.scalar.activation(z_sb[:, 2 * jp:2 * jp + 2, :],
                                                 zps[:, :, 0:C + 1], Copy)
                        else:
                            nc.vector.tensor_copy(z_sb[:, 2 * jp:2 * jp + 2, :],
                                                  zps[:, :, 0:C + 1])
                    nc.sync.dma_start(out=zs[b, :, g * 8:(g + 1) * 8, :], in_=z_sb[:])
    nc.compile()
    return nc


def _prep_inputs(x, w_qkv, w_proj, bn_gamma, bn_beta, bn_mean, bn_var):
    x = np.asarray(x, dtype=np.float32)
    w_qkv = np.asarray(w_qkv, dtype=np.float32)
    w_proj = np.asarray(w_proj, dtype=np.float32)
    bn_gamma = np.asarray(bn_gamma, dtype=np.float32)
    bn_beta = np.asarray(bn_beta, dtype=np.float32)
    bn_mean = np.asarray(bn_mean, dtype=np.float32)
    bn_var = np.asarray(bn_var, dtype=np.float32)

    # torch-faithful interleave: out-channel 3*i+j -> (channel i, {q,k,v}[j])
    def w8(wm):  # (C_out, C_in) -> [128, 2, C_out] fp8: [p, j, o] = w[o, j*128+p]
        return np.ascontiguousarray(
            wm.T.reshape(2, 128, -1).transpose(1, 0, 2).astype(NPFP8))

    wq8 = w8(w_qkv[0::3])
    # combined [wk | wv] along the output dim
    wkv8 = w8(np.concatenate([w_qkv[1::3], w_qkv[2::3]], axis=0))
    scale = bn_gamma / np.sqrt(bn_var + 1e-5)
    wp_t = np.ascontiguousarray((scale[:, None] * w_proj).T.astype(NPBF16))
    # x: (B, C, N) -> [B, 128, 2, N] fp8: [b, p, j, n] = x[b, j*128+p, n]
    x8 = np.ascontiguousarray(
        x.reshape(B, 2, 128, N).transpose(0, 2, 1, 3).astype(NPFP8))

    bias = (bn_beta - bn_mean * scale).astype(np.float32)

    in_maps = []
    for core in range(NCORES):
        in_maps.append({
            "x8": x8[core * BL:(core + 1) * BL],
            "wq8": wq8, "wkv8": wkv8, "wpt": wp_t,
        })
    return in_maps, bias


def _postprocess(z_raw, bias):
    # z_raw: (B, 128, NT, C+1) f32, n = i*128+p -> y (B, C, H, W) f32
    z = z_raw.transpose(0, 2, 1, 3).reshape(B, N, C + 1)
    y = z[:, :, :C] / z[:, :, C:C + 1] + bias[None, None, :]
    return np.ascontiguousarray(y.transpose(0, 2, 1)).reshape(B, C, H, W)


def _run(inputs, trace=False, **kw):
    if "nc" not in _CACHE:
        _CACHE["nc"] = _build_program()
    nc = _CACHE["nc"]
    in_maps, bias = _prep_inputs(**inputs)
    res = run_bass_kernel_spmd(nc, in_maps, list(range(NCORES)), trace=trace, **kw)
    z_raw = np.concatenate([res.results[i]["z"] for i in range(NCORES)], axis=0)
    return _postprocess(z_raw.astype(np.float32), bias), res


def kernel(**inputs):
    y, _ = _run(inputs)
    return y
